# revision 92
# baseline (speedup 1.0000x reference)
"""Trainium2 Bass kernel for nn_Attention_78151224918608.

Dense transformer attention block: QKV proj + RoPE + GQA causal attention
+ output proj. Sharding: hybrid (batch x head-group) across 8 cores:
core c = (b, g) with b = c//4, g = c%4 handles batch b, q-heads
8g..8g+7, kv-heads 2g, 2g+1. Each core emits a full-width [S, D] bf16
partial (its heads through their wo rows); host sums 4 partials per
batch in fp32 and casts to bf16.

Per-core layout (all bf16, fp32 PSUM accumulation):
  - QK^T projections computed transposed (head-dim on partitions) in five
    128-row m-blocks [Q01 Q23 Q45 Q67 | K(2 kv)] so RoPE runs on full
    128-partition tiles and scores lhsT/rhs slices line up.
  - V projected token-major (tokens on partitions) per 128-token block:
    out[tok, v] with lhsT = x^T block, rhs = wv. Lands exactly in the PV
    rhs layout [128 tok, kt, kv, 65] (col 64 = ones for sumexp) with no
    transpose.
  - RoPE pair-swap folded into wq/wk columns (evens-then-odds perm); the
    swap becomes 32-row block copies (DVE 4x) + two mults + add (DVE 2x).
  - Scores computed transposed per (h, qtile): S^T[k, q] strips of 2
    k-tiles in PSUM [128, 1024]. Diagonal k-tiles compute only their
    valid q-suffix; the causal triangle is zeroed post-exp by a [128,128]
    0/1 multiply on DVE. One PSUM accumulation group open per bank at a
    time (hardware corrupts interleaved open groups within a bank).
  - exp on ScalarE reads PSUM strips, applies the 1/sqrt(hd) softmax
    scale as an immediate, writes bf16 SBUF. ScalarE does nothing else
    during attention - it is the second-busiest engine.
  - PV token-major per 128-token q-block: O[q, 0:64] and sumexp in col
    64 via the ones column of V; e-strips are retained per head and each
    q-block's accumulation chain runs open->close sequentially.
    Normalization is reciprocal + per-partition tensor_scalar (DVE) into
    head-pair tiles [128 q, 128]; a DMA-engine xbar transpose lands each
    pair directly in O^T layout (no PE, no PSUM; the very last head uses
    a PE transpose to dodge DMA latency on the critical tail).
  - wo: O^T[128, 4, S] @ wo[512, D] in [128,512] PSUM slabs, evacuated
    by DVE (plus ScalarE during the final flush), DMA'd per row block.
  - Software pipelining: projections are split into drainable units
    (K/V/Q-pair per 512-token chunk) and wo into per-512-column slabs;
    both are interleaved into the attention head loops as PE filler
    wherever ScalarE's exp stream would otherwise stall the in-order PE,
    with wo slabs hoarded for the final (most exp-bound) q-tile.
"""

import sys

sys.path.insert(0, "/opt/trn_rl_repo")

import math
import numpy as np
import ml_dtypes

BF16 = ml_dtypes.bfloat16

# Problem constants (hardcoded per contract).
B = 2
S = 2048
D = 2048
N_HEADS = 32
N_KV_HEADS = 8
HD = 64
N_CORES = 8
HQ = N_HEADS // 4  # 8 q heads per core (4 head-groups)
KV = N_KV_HEADS // 4  # 2 kv heads per core
M_QK = HQ * HD + KV * HD  # 640: [Q pairs x4 | K]
M_ALL = M_QK + KV * HD  # 768 incl. V cols in wqkv dram tensor
QTS = 512  # q tile size (free dim)
KTS = 128  # k tile size (partitions)
GRP = 2  # k-tiles per score strip
SM_SCALE = 1.0 / math.sqrt(HD)


def build_program(s=S, d=D, phase_log=None):
    import concourse.bass as bass
    import concourse.mybir as mybir
    import concourse.tile as tile
    from concourse import bacc

    def mark(label):
        if phase_log is not None:
            phase_log.append((label, len(nc.inst_map)))

    f32 = mybir.dt.float32
    bf16 = mybir.dt.bfloat16
    Exp = mybir.ActivationFunctionType.Exp
    Copy = mybir.ActivationFunctionType.Copy
    add_op = mybir.AluOpType.add
    mult_op = mybir.AluOpType.mult

    n_qt = s // QTS  # 4 q tiles
    n_dkt = d // 128  # 16 contraction tiles for projections
    n_skt = s // KTS  # 16 k tiles
    n_nt = s // QTS  # 4 token tiles for proj free dim
    n_tb = s // 128  # 16 token blocks for V / wo rows
    n_mo = (HQ * HD) // 128  # 4 wo contraction tiles

    nc = bacc.Bacc("TRN2", num_devices=N_CORES)
    # host pre-tiles to [128, kt, ...] so each load is a single strided DMA
    xT_d = nc.declare_dram_parameter("xT", [128, d // 128, s], bf16, isOutput=False)
    wqkv_d = nc.declare_dram_parameter(
        "wqkv", [128, d // 128, M_ALL], bf16, isOutput=False
    )
    wo_d = nc.declare_dram_parameter(
        "wo_s", [128, (HQ * HD) // 128, d], bf16, isOutput=False
    )
    cos_d = nc.declare_dram_parameter("cosb", [128, s], bf16, isOutput=False)
    sin_d = nc.declare_dram_parameter("sinb", [128, s], bf16, isOutput=False)
    tri_d = nc.declare_dram_parameter("trimask", [128, 128], bf16, isOutput=False)
    tri01_d = nc.declare_dram_parameter("tri01", [128, 128], bf16, isOutput=False)
    id_d = nc.declare_dram_parameter("ident", [128, 128], bf16, isOutput=False)
    part_d = nc.declare_dram_parameter("part", [s, d], bf16, isOutput=True)

    with tile.TileContext(nc) as tc:
        with (
            tc.tile_pool(name="const", bufs=1) as cpool,
            tc.tile_pool(name="big", bufs=1) as bpool,
            tc.tile_pool(name="work", bufs=3) as wpool,
            tc.tile_pool(name="small", bufs=8) as spool,
            tc.tile_pool(name="estrip", bufs=10) as epool,
            tc.tile_pool(name="outp", bufs=3) as opool,
            tc.tile_pool(name="pssc", bufs=2, space="PSUM") as pssc,
            tc.tile_pool(name="psops", bufs=1, space="PSUM") as psops,
            tc.tile_pool(name="pswo", bufs=2, space="PSUM") as pswo,
        ):
            # ---- constants / weights ----
            cos_sb = cpool.tile([128, s], bf16)
            sin_sb = cpool.tile([128, s], bf16)
            tri_sb = cpool.tile([128, 128], bf16)
            tri01_sb = cpool.tile([128, 128], bf16)
            id_sb = cpool.tile([128, 128], bf16)
            wqkv_sb = cpool.tile([128, n_dkt, M_ALL], bf16)
            wo_sb = cpool.tile([128, n_mo, d], bf16)
            xT_sb = cpool.tile([128, n_dkt, s], bf16)

            # DMA order tuned for the pipeline: x chunk 0 + K-block weights
            # first (first proj matmuls), then Q weights, rope tables, the
            # rest of x, and everything only needed later.
            nc.sync.dma_start(
                wqkv_sb[:, 0:4, 512:M_ALL], wqkv_d[:, 0:4, 512:M_ALL]
            )  # K + V cols, first contraction quarter
            nc.sync.dma_start(xT_sb[:, 0:4, 0:QTS], xT_d[:, 0:4, 0:QTS])
            nc.sync.dma_start(
                wqkv_sb[:, 4:16, 512:M_ALL], wqkv_d[:, 4:16, 512:M_ALL]
            )
            for kq in range(1, 4):  # rest of x chunk 0 in quarters
                ksl = slice(kq * 4, (kq + 1) * 4)
                nc.sync.dma_start(xT_sb[:, ksl, 0:QTS], xT_d[:, ksl, 0:QTS])
            nc.sync.dma_start(wqkv_sb[:, :, 0:128], wqkv_d[:, :, 0:128])  # Q0
            nc.sync.dma_start(wqkv_sb[:, :, 128:512], wqkv_d[:, :, 128:512])
            nc.sync.dma_start(cos_sb[:], cos_d[:])
            nc.sync.dma_start(sin_sb[:], sin_d[:])
            nc.sync.dma_start(tri_sb[:], tri_d[:])
            nc.sync.dma_start(tri01_sb[:], tri01_d[:])
            nc.sync.dma_start(id_sb[:], id_d[:])
            for n in range(1, n_nt):
                nsl = slice(n * QTS, (n + 1) * QTS)
                nc.sync.dma_start(xT_sb[:, :, nsl], xT_d[:, :, nsl])
            nc.sync.dma_start(wo_sb[:], wo_d[:])

            # ---- persistent per-core tiles ----
            QT_sb = bpool.tile([128, HQ // 2, s], bf16)  # 4 pair-blocks
            KT2_sb = bpool.tile([128, KV, s], bf16)  # kv duplicated to both hb
            V_sb = bpool.tile([128, n_skt, KV, 65], bf16)  # token-major + ones
            OT_sb = bpool.tile([128, n_mo, s], bf16)

            def proj_qk(n, m):
                """One 128-row QK^T m-block for tokens n*512..: matmuls+rope.
                m in 0..3 = Q pair-blocks, m == 4 = K (2 kv heads).
                Alternates between the two 1-bank PSUM rings so consecutive
                units pipeline."""
                nsl = slice(n * QTS, (n + 1) * QTS)
                if m % 2 == 0:
                    ps = psops.tile([128, QTS], f32, tag="trpj", name="pjps")
                else:
                    ps = pswo.tile([128, QTS], f32, tag="wo", name="pjps2")
                for kt in range(n_dkt):
                    nc.tensor.matmul(
                        ps[:, 0:QTS],
                        wqkv_sb[:, kt, m * 128 : (m + 1) * 128],
                        xT_sb[:, kt, nsl],
                        start=(kt == 0),
                        stop=(kt == n_dkt - 1),
                    )
                q_raw = wpool.tile([128, QTS], bf16, tag="qraw")
                nc.scalar.activation(q_raw[:], ps[:, 0:QTS], Copy)
                t1 = wpool.tile([128, QTS], bf16, tag="t1")
                t2 = wpool.tile([128, QTS], bf16, tag="t2")
                qsw = wpool.tile([128, QTS], bf16, tag="qsw")
                nc.vector.tensor_tensor(t1[:], q_raw[:], cos_sb[:, nsl], mult_op)
                for r0, r1 in ((0, 32), (32, 0), (64, 96), (96, 64)):
                    nc.vector.tensor_copy(
                        qsw[r0 : r0 + 32, :], q_raw[r1 : r1 + 32, :]
                    )
                nc.vector.tensor_tensor(t2[:], qsw[:], sin_sb[:, nsl], mult_op)
                if m < HQ // 2:
                    nc.vector.tensor_tensor(QT_sb[:, m, nsl], t1[:], t2[:], add_op)
                else:
                    kt_s = wpool.tile([128, QTS], bf16, tag="kts")
                    nc.vector.tensor_tensor(kt_s[:], t1[:], t2[:], add_op)
                    # duplicate each kv head across both 64-row halves
                    for v in range(KV):
                        src = kt_s[v * 64 : (v + 1) * 64, :]
                        nc.vector.tensor_copy(KT2_sb[0:64, v, nsl], src)
                        nc.vector.tensor_copy(KT2_sb[64:128, v, nsl], src)

            def proj_v(n, tb):
                """One token-major V block (128 tokens, both kv heads)."""
                tsl = slice(tb * 128, (tb + 1) * 128)
                vps = pswo.tile([128, KV * HD], f32, tag="wo")
                for kt in range(n_dkt):
                    nc.tensor.matmul(
                        vps[:],
                        xT_sb[:, kt, tsl],
                        wqkv_sb[:, kt, M_QK:M_ALL],
                        start=(kt == 0),
                        stop=(kt == n_dkt - 1),
                    )
                for v in range(KV):
                    nc.vector.tensor_copy(
                        V_sb[:, tb, v, 0:64], vps[:, v * 64 : (v + 1) * 64]
                    )
                    nc.gpsimd.memset(V_sb[:, tb, v, 64:65], 1.0)

            # proj work as drainable unit queues, interleaved into attention.
            # Order: K, Q0 (longest DVE rope chains first), V x4 (PE filler
            # while rope runs), then remaining Q pair-blocks. For n == 0 the
            # V blocks go before Q0 so PE has work while the Q-column DMA is
            # still in flight.
            proj_units = {
                n: [(0, lambda n=n, m=HQ // 2: proj_qk(n, m))]
                + (
                    [(0, lambda n=n, tb=tb: proj_v(n, tb)) for tb in range(4)]
                    + [(0, lambda n=n, m=0: proj_qk(n, m))]
                    if n == 0
                    else [(0, lambda n=n, m=0: proj_qk(n, m))]
                    + [
                        (0, lambda n=n, tb=tb: proj_v(n, tb))
                        for tb in range(4 * n, 4 * n + 4)
                    ]
                )
                + [(pb, lambda n=n, m=pb: proj_qk(n, m)) for pb in range(1, HQ // 2)]
                for n in range(n_nt)
            }

            def ensure_proj(n, pb):
                """Drain proj(n) units required before head pair pb runs."""
                q = proj_units[n]
                while q and q[0][0] <= pb:
                    q.pop(0)[1]()

            def drain_proj(n, k=1):
                q = proj_units.get(n)
                for _ in range(k):
                    if q:
                        q.pop(0)[1]()

            # ---- attention ----
            o2_cur = [None] * 4  # per-qb [128 q, 128] pair tiles (2 heads)

            def normalize(h, qt, ops):
                """Reciprocal + per-partition scale (DVE) into a head-pair
                tile; at odd heads one DMA-engine transpose per q-block lands
                the pair directly in O^T layout (no PE, no PSUM)."""
                pb, hb = h // 2, (h % 2) * 64
                for qb in range(4):
                    rt = spool.tile([128, 1], f32, tag="rt")
                    nc.vector.reciprocal(rt[:], ops[:, qb, 64:65])
                    if h % 2 == 0:
                        o2_cur[qb] = wpool.tile(
                            [128, 128], bf16, tag="osb_n", bufs=8, name="o2"
                        )
                    nc.vector.tensor_scalar(
                        o2_cur[qb][:, hb : hb + 64], ops[:, qb, 0:64],
                        rt[:], None, mult_op,
                    )
                    if h % 2 == 1:
                        osl = slice(
                            qt * QTS + qb * 128, qt * QTS + (qb + 1) * 128
                        )
                        if qt == n_qt - 1 and h == HQ - 1:
                            # last head: PE transpose beats the DMA-engine
                            # round-trip latency on the critical tail
                            tr = psops.tile(
                                [128, 128], bf16, tag="trpj", name="tr"
                            )
                            nc.tensor.transpose(tr[:], o2_cur[qb][:], id_sb[:])
                            nc.vector.tensor_copy(OT_sb[:, pb, osl], tr[:])
                        else:
                            nc.sync.dma_start_transpose(
                                OT_sb[:, pb, osl], o2_cur[qb][:]
                            )

            def attn(qt):
                mark(f"attn{qt}")
                n_kt = 4 * (qt + 1)
                qbase = qt * QTS
                for h in range(HQ):
                    pb, hb = h // 2, (h % 2) * 64
                    v = h // 4
                    ensure_proj(qt, pb)
                    qh = QT_sb[hb : hb + 64, pb, :]
                    kt2 = KT2_sb[hb : hb + 64, v, :]
                    ops = psops.tile([128, 4, 65], f32, tag="ops")
                    e_strips = []
                    for g in range(0, n_kt, GRP):
                        kts = (g, g + 1)
                        sc = pssc.tile([128, GRP * QTS], f32, tag="sc")
                        o0 = None
                        for j, kt in enumerate(kts):
                            o = kt * KTS - qbase
                            if o0 is None:
                                o0 = max(0, o)
                            lh = kt2[:, kt * KTS : (kt + 1) * KTS]
                            if o < 0:  # fully valid tile
                                nc.tensor.matmul(
                                    sc[:, j * QTS : (j + 1) * QTS],
                                    lh,
                                    qh[:, qbase : qbase + QTS],
                                    start=True,
                                    stop=True,
                                )
                            else:  # diagonal: compute the valid q-suffix only;
                                # the triangle is zeroed post-exp on DVE
                                nc.tensor.matmul(
                                    sc[:, j * QTS + o : (j + 1) * QTS],
                                    lh,
                                    qh[:, qbase + o : qbase + QTS],
                                    start=True,
                                    stop=True,
                                )
                        if g % 4 == 0:
                            # drain own remaining units first (hides their
                            # rope latency), then feed the next q-tile's
                            if proj_units[qt]:
                                drain_proj(qt, 1)
                            elif qt + 1 < n_nt:
                                drain_proj(qt + 1, 1)
                        e = epool.tile([128, GRP * QTS], bf16, tag="e")
                        nc.scalar.activation(
                            e[:, o0 : GRP * QTS], sc[:, o0 : GRP * QTS],
                            Exp, scale=SM_SCALE,
                        )
                        for j, kt in enumerate(kts):
                            o = kt * KTS - qbase
                            if o >= 0:  # zero the diag block's upper triangle
                                nc.vector.tensor_tensor(
                                    e[:, j * QTS + o : j * QTS + o + 128],
                                    e[:, j * QTS + o : j * QTS + o + 128],
                                    tri01_sb[:],
                                    mult_op,
                                )
                        e_strips.append((e, kts))
                        # final q-tile: spread wo slabs between strips so PE
                        # never starves while ScalarE works through the exps
                        # (48 slabs over 8 heads -> skip 2 of 8 slots/head)
                        if qt == n_qt - 1 and not (h < 4 and g in (2, 6, 14)):
                            wo_drain(1, borrow=(h < HQ - 1))
                    # PV token-major, one q-block at a time so each PSUM
                    # accumulation group opens and closes without another
                    # group interleaving in the same bank
                    for qb in range(4):
                        g_qb = 4 * qt + qb
                        for e, kts in e_strips:
                            for j, kt in enumerate(kts):
                                if kt > g_qb:
                                    continue
                                nc.tensor.matmul(
                                    ops[:, qb, :],
                                    e[:, j * QTS + qb * 128 : j * QTS + (qb + 1) * 128],
                                    V_sb[:, kt, v, :],
                                    start=(kt == 0),
                                    stop=(kt == g_qb),
                                )
                    normalize(h, qt, ops)

            wo_queue = []

            wo_state = {"osb": None, "n": 0}

            def wo_unit(mt, ncol, act_evac=False, borrow=False):
                """One [128 tok, 512 dcol] slab of the output projection.
                borrow=True rotates through the (idle) trpj bank as a third
                PSUM slot so DVE evac latency never stalls the slab chain."""
                msl = slice(mt * 128, (mt + 1) * 128)
                if ncol == 0:
                    wo_state["osb"] = opool.tile(
                        [128, d], bf16, tag="osb", name="osb"
                    )
                osb = wo_state["osb"]
                nsl = slice(ncol * QTS, (ncol + 1) * QTS)
                wo_state["n"] += 1
                if borrow and wo_state["n"] % 3 == 0:
                    ps = psops.tile([128, QTS], f32, tag="trpj", name="wops")
                else:
                    ps = pswo.tile([128, QTS], f32, tag="wo")
                for kt in range(n_mo):
                    nc.tensor.matmul(
                        ps[:],
                        OT_sb[:, kt, msl],
                        wo_sb[:, kt, nsl],
                        start=(kt == 0),
                        stop=(kt == n_mo - 1),
                    )
                if act_evac and ncol % 2 == 1:
                    nc.scalar.activation(osb[:, nsl], ps[:], Copy)
                else:
                    nc.vector.tensor_copy(osb[:, nsl], ps[:])
                if act_evac:
                    # flush path: stream each slab out as soon as it lands
                    nc.sync.dma_start(
                        part_d[mt * 128 : (mt + 1) * 128, nsl], osb[:, nsl]
                    )
                elif ncol == d // QTS - 1:
                    nc.sync.dma_start(
                        part_d[mt * 128 : (mt + 1) * 128, :], osb[:]
                    )

            def wo_drain(k=1, borrow=False):
                for _ in range(k):
                    if wo_queue:
                        wo_unit(*wo_queue.pop(0), borrow=borrow)

            for qt in range(n_qt):
                attn(qt)  # drains proj units + wo slabs as it goes
                wo_queue.extend(
                    (mt, ncol)
                    for mt in range(4 * qt, 4 * qt + 4)
                    for ncol in range(d // QTS)
                )
            # flush remaining wo slabs (ScalarE helps evacuate: no exps left)
            while wo_queue:
                wo_unit(*wo_queue.pop(0), act_evac=True)
    mark("end")
    nc.compile()
    return nc


# ---------------- host-side sharding ----------------

_PERM = np.concatenate([np.arange(0, HD, 2), np.arange(1, HD, 2)])  # evens, odds


def _ktile(a, d0=128):
    """[K, N] -> [128, K//128, N] (partition-major k-tiling)."""
    k, n = a.shape
    return np.ascontiguousarray(
        a.reshape(k // d0, d0, n).transpose(1, 0, 2)
    )


def make_core_inputs(x, freqs_cos, freqs_sin, wq, wk, wv, wo, s=S, d=D):
    """Build per-core input maps: core c = (b=c//4, g=c%4)."""
    xT = [_ktile(x[b].T.astype(BF16)) for b in range(B)]

    cosT = np.ascontiguousarray(freqs_cos.T)  # [32, S]
    sinT = np.ascontiguousarray(freqs_sin.T)
    cosb = np.tile(np.concatenate([cosT, cosT], axis=0), (2, 1)).astype(BF16)
    sinb = np.tile(np.concatenate([-sinT, sinT], axis=0), (2, 1)).astype(BF16)

    k = np.arange(128)[:, None]
    c = np.arange(128)[None, :]
    trimask = np.where(c >= k, 0.0, -1e9).astype(BF16)
    tri01 = np.where(c >= k, 1.0, 0.0).astype(BF16)
    ident = np.eye(128).astype(BF16)

    in_maps = []
    for core in range(N_CORES):
        b, g = core // 4, core % 4
        wq_c = np.concatenate(
            [
                wq[:, (HQ * g + h) * HD : (HQ * g + h + 1) * HD][:, _PERM]
                for h in range(HQ)
            ],
            axis=1,
        )  # [D, 512]
        wk_c = np.concatenate(
            [
                wk[:, (KV * g + v) * HD : (KV * g + v + 1) * HD][:, _PERM]
                for v in range(KV)
            ],
            axis=1,
        )  # [D, 128]
        wv_c = wv[:, KV * g * HD : KV * (g + 1) * HD]  # [D, 128]
        wqkv = _ktile(np.concatenate([wq_c, wk_c, wv_c], axis=1).astype(BF16))
        wo_c = _ktile(wo[HQ * g * HD : HQ * (g + 1) * HD, :].astype(BF16))
        in_maps.append(
            {
                "xT": xT[b],
                "wqkv": wqkv,
                "wo_s": wo_c,
                "cosb": cosb,
                "sinb": sinb,
                "trimask": trimask,
                "tri01": tri01,
                "ident": ident,
            }
        )
    return in_maps


_NC_CACHE = {}


def kernel(x, freqs_cos, freqs_sin, wq, wk, wv, wo):
    from concourse.bass_utils import run_bass_kernel_spmd

    x = np.asarray(x, np.float32)
    freqs_cos = np.asarray(freqs_cos, np.float32)
    freqs_sin = np.asarray(freqs_sin, np.float32)
    wq = np.asarray(wq, np.float32)
    wk = np.asarray(wk, np.float32)
    wv = np.asarray(wv, np.float32)
    wo = np.asarray(wo, np.float32)

    if "nc" not in _NC_CACHE:
        _NC_CACHE["nc"] = build_program()
    nc = _NC_CACHE["nc"]

    in_maps = make_core_inputs(x, freqs_cos, freqs_sin, wq, wk, wv, wo)
    res = run_bass_kernel_spmd(nc, in_maps, list(range(N_CORES)))
    out = np.zeros((B, S, D), np.float32)
    for core in range(N_CORES):
        out[core // 4] += np.asarray(res.results[core]["part"], np.float32)
    return out.astype(BF16)
